# revision 11
# baseline (speedup 1.0000x reference)
"""Trainium2 Bass kernel for nn_AttentionSimple (sparse_attention, 8 cores).

Reference (per batch row b):
    e      = embeddings[k[b]]              # [S, E] gather
    scores = q[b] . e[s]                   # [S]
    attn   = softmax(scores); ctx = sum_s attn[s] * e[s]
    out    = ctx @ W.T + b                 # [B, 2]

Algorithm: count-weighted vocab-space softmax — no per-token gathers.
Scores depend on s only through v = k[b, s], so group softmax terms by
vocabulary id:
    c[b, v]  = |{s : k[b, s] = v}|         (histogram of k, built on host)
    l[b, v]  = q[b] . embeddings[v]        (dense PE matmul)
    A        = c * exp(l - 30)             (global bias keeps exp in fp16;
                                            the ratio is shift-invariant)
    out[b]   = (sum_v A[b,v] * EW[v]) / (sum_v A[b,v])
    with EW  = embeddings @ W.T + b        (parameter prepacking, host)

Sharding: padded vocabulary 51200 = 400 chunks of 128, 50 chunks/core.
Each core handles all 128 batch rows for its slice; host sums the 8
partial numerators/denominators and divides.

v2 pipeline (per core), all wire data fp16 except uint8 counts:
  - et: embedding pairs packed at partition rows 0:50 / 50:100 (no
    zero-pad rows on the wire), [100, 25*128] fp16 — half the f32 bytes.
  - mm1: per pair, ps[128v, 256] = et_pair.T @ qw (qw = block-diag
    [qT|0; 0|qT] fp16, [100, 256]); 4 pairs fill a [128, 1024] 2-bank
    PSUM block (2 quads).
  - ACT: le = exp(ps - 30) fused PSUM->SBUF, fp16 out, one 1024-col
    instruction per block (amortizes the ~250ns fixed ACT overhead).
  - DVE: le *= counts (uint8, exact; max count 5) in place, 1024 cols.
  - mm2: acc[9, 512] += st9_quad.T @ le_quad (fp16, f32 PSUM accum);
    st9 = [EW_c0..EW_c3 | ones] per quad; 13 accumulating matmuls.
  - 50 chunks = 12 full quads + 1 half quad (chunks 48,49); the half
    block's unused PSUM cols never feed mm2 (rhs is narrowed to 256).
  - Warm-up matmuls ramp the PE p-state while the first DMAs land.
  - Inputs live in single resident SBUF tiles; DMAs are column-sliced
    so compute only waits on the slice it reads (subtile deps).  et
    slices ride the Sync queue, qw/st/ct ride the Scalar queue, so
    issue cost is split across two queues.
  - Output: acc PSUM is DMA'd straight to DRAM (no SBUF copy).
"""

import numpy as np

BATCH, SEQ, EMB, VOCAB, OUT = 128, 8192, 50, 50000, 2
N_CORES = 8
CSH = 50                         # vocab chunks per core
NCHUNK = CSH * N_CORES           # 400
VPAD = NCHUNK * 128              # 51200
VSH = CSH * 128                  # 6400
NPAIR = CSH // 2                 # 25
NQUAD = 13                       # 12 full + 1 half
NBLOCK = 7                       # 6 full (4 pairs) + 1 micro (1 pair)
EXP_BIAS = -30.0
NWARM = 3

_CACHE = {}


def _build_nc():
    from contextlib import ExitStack

    import concourse.mybir as mybir
    import concourse.tile as tile
    from concourse import bacc

    f32 = mybir.dt.float32
    f16 = mybir.dt.float16
    u8 = mybir.dt.uint8
    nc = bacc.Bacc("TRN2", target_bir_lowering=False, debug=False,
                   num_devices=N_CORES)

    et_d = nc.dram_tensor("et", [100, NPAIR * 128], f16, kind="ExternalInput")
    qw_d = nc.dram_tensor("qw", [100, 256], f16, kind="ExternalInput")
    st_d = nc.dram_tensor("st", [128, NQUAD * 9], f16, kind="ExternalInput")
    ct_d = nc.dram_tensor("ct", [128, VSH], u8, kind="ExternalInput")
    o_d = nc.dram_tensor("o", [18, 512], f32, kind="ExternalOutput")

    with tile.TileContext(nc) as tc, ExitStack() as ctx:
        const_p = ctx.enter_context(tc.tile_pool(name="const", bufs=1))
        ps_p = ctx.enter_context(tc.tile_pool(name="ps", bufs=3, space="PSUM"))
        acc_p = ctx.enter_context(tc.tile_pool(name="acc", bufs=1,
                                               space="PSUM"))
        le_p = ctx.enter_context(tc.tile_pool(name="le", bufs=3))

        # Two accumulators: accA (quads 0-5) finalizes mid-kernel so its
        # copy + output DMA overlap the remaining blocks; accB takes the
        # rest.  Warm-up matmuls write into accB's bank (start=True on the
        # first real accB matmul resets it), keeping PSUM within 8 banks.
        accA = acc_p.tile([9, 512], f32, tag="accA")
        accBf = acc_p.tile([128, 512], f32, tag="accB")
        accB = accBf[0:9, :]

        # PE warm-up: matmuls on a zeroed tile while input DMAs land, so
        # the PE p-state ramp starts as early as possible.
        wtile = const_p.tile([128, 512], f16)
        nc.vector.memset(wtile[:], 0.0)
        bias_sb = const_p.tile([128, 1], f32)
        nc.vector.memset(bias_sb[:], EXP_BIAS)
        for _ in range(NWARM):
            nc.tensor.matmul(accBf[:], lhsT=wtile[:, 0:128], rhs=wtile[:],
                             start=True, stop=True, skip_group_check=True)

        # Resident input tiles; column-sliced DMAs in consumption order.
        # et + qw ride the Sync queue, ct + st ride the Vector queue, so
        # issue cost is split and the Scalar queue stays free for exp.
        qw_sb = const_p.tile([100, 256], f16)
        nc.sync.dma_start(qw_sb[:], qw_d.ap())
        st_sb = const_p.tile([128, NQUAD * 9], f16)
        nc.scalar.dma_start(st_sb[:], st_d.ap())
        et_sb = const_p.tile([100, NPAIR * 128], f16)
        ct_sb = const_p.tile([128, VSH], u8)
        for c0, c1 in ((0, 512), (512, 1536), (1536, 2560), (2560, 3200)):
            nc.sync.dma_start(et_sb[:, c0:c1], et_d.ap()[:, c0:c1])
        for c0, c1 in ((0, 1024), (1024, 3072), (3072, 5120), (5120, VSH)):
            nc.scalar.dma_start(ct_sb[:, c0:c1], ct_d.ap()[:, c0:c1])

        osbA = const_p.tile([9, 512], f32)
        osbB = const_p.tile([9, 512], f32)

        for blk in range(NBLOCK):
            micro = blk == NBLOCK - 1
            npair = 1 if micro else 4
            ncol = 256 * npair
            ps = ps_p.tile([128, 1024], f32, tag="ps")
            for lp in range(npair):
                pair = 4 * blk + lp
                nc.tensor.matmul(
                    ps[:, lp * 256:(lp + 1) * 256],
                    lhsT=et_sb[:, pair * 128:(pair + 1) * 128],
                    rhs=qw_sb[:],
                    start=True, stop=True,
                )
            le = le_p.tile([128, 1024], f16, tag="le")
            nc.scalar.activation(le[:, 0:ncol], ps[:, 0:ncol],
                                 mybir.ActivationFunctionType.Exp,
                                 bias=bias_sb[:])
            nc.vector.tensor_mul(
                le[:, 0:ncol], le[:, 0:ncol],
                ct_sb[:, blk * 1024:blk * 1024 + ncol])
            for lq in range(1 if micro else 2):
                quad = 2 * blk + lq
                acc = accA if quad < 6 else accB
                nc.tensor.matmul(
                    acc[:, 0:256] if micro else acc,
                    lhsT=st_sb[:, quad * 9:(quad + 1) * 9],
                    rhs=le[:, lq * 512:lq * 512 + (256 if micro else 512)],
                    start=(quad in (0, 6)), stop=(quad in (5, NQUAD - 1)),
                    skip_group_check=True,
                )
            if blk == 2:
                # accA finalized (quads 0-5): ship it while blocks 3-6 run.
                nc.scalar.copy(osbA[:], accA)
                nc.sync.dma_start(o_d.ap()[0:9, :], osbA[:])

        nc.scalar.copy(osbB[:], accB)
        nc.sync.dma_start(o_d.ap()[9:18, :], osbB[:])

    nc.finalize()
    return nc


def _prep_inputs(q, k, embeddings, W, b):
    q = np.ascontiguousarray(q, dtype=np.float32)
    emb = np.ascontiguousarray(embeddings, dtype=np.float32)
    W = np.ascontiguousarray(W, dtype=np.float32)
    b = np.ascontiguousarray(b, dtype=np.float32)
    k = np.asarray(k)

    embT = np.zeros((EMB, VPAD), np.float32)
    embT[:, :VOCAB] = emb.T

    # mm1 moving operand: block-diagonal [qT | 0; 0 | qT], rows 0:50/50:100
    qw = np.zeros((100, 256), np.float16)
    qw[:EMB, 0:BATCH] = q.T
    qw[EMB:2 * EMB, BATCH:256] = q.T

    # weight prepacking: EW = emb @ W.T + b (function of parameters only)
    EWp = np.zeros((VPAD, OUT), np.float32)
    EWp[:VOCAB] = emb @ W.T + b[None, :]

    flat = (np.arange(BATCH, dtype=np.int64)[:, None] * VPAD
            + k.astype(np.int64)).ravel()
    C = np.bincount(flat, minlength=BATCH * VPAD).reshape(BATCH, VPAD)
    assert C.max() <= 255, "count histogram overflows uint8 transport"

    in_maps = []
    for core in range(N_CORES):
        v0 = core * VSH
        blocks = embT[:, v0:v0 + VSH].reshape(EMB, CSH, 128)
        e2 = np.zeros((100, NPAIR, 128), np.float16)
        e2[:EMB] = blocks[:, 0::2, :]
        e2[EMB:2 * EMB] = blocks[:, 1::2, :]
        e2 = np.ascontiguousarray(e2.reshape(100, NPAIR * 128))

        # st9 per quad: cols 2j+o = EW[chunk 4q+j, p, o]; col 8 = 1
        ew_blocks = EWp[v0:v0 + VSH].reshape(CSH, 128, OUT)  # [50, 128, 2]
        st = np.zeros((128, NQUAD, 9), np.float32)
        for quad in range(NQUAD):
            for j in range(4):
                ch = 4 * quad + j
                if ch < CSH:
                    st[:, quad, 2 * j:2 * j + 2] = ew_blocks[ch]
        st[:, :, 8] = 1.0
        st = np.ascontiguousarray(
            st.reshape(128, NQUAD * 9).astype(np.float16))

        ct = np.ascontiguousarray(
            C[:, v0:v0 + VSH].reshape(BATCH, CSH, 128)
            .transpose(2, 1, 0).reshape(128, CSH * BATCH)
            .astype(np.uint8))
        in_maps.append({"et": e2, "qw": qw, "st": st, "ct": ct})
    return in_maps


def _run_device(in_maps, **kwargs):
    from concourse.bass_utils import run_bass_kernel_spmd

    if "nc" not in _CACHE:
        _CACHE["nc"] = _build_nc()
    return run_bass_kernel_spmd(_CACHE["nc"], in_maps,
                                core_ids=list(range(N_CORES)), **kwargs)


def _unshard(res):
    P = np.zeros((9, 512), np.float64)
    for i in range(N_CORES):
        o = res.results[i]["o"].astype(np.float64)
        P += o[0:9] + o[9:18]
    numer = np.zeros((OUT, BATCH), np.float64)
    denom = np.zeros(BATCH, np.float64)
    for j in range(4):
        numer += P[2 * j:2 * j + 2, j * BATCH:(j + 1) * BATCH]
        denom += P[8, j * BATCH:(j + 1) * BATCH]
    out = (numer / denom[None, :]).T
    return np.ascontiguousarray(out, dtype=np.float32)


def kernel(q, k, embeddings, W, b, **_unused):
    in_maps = _prep_inputs(q, k, embeddings, W, b)
    res = _run_device(in_maps)
    return _unshard(res)


# revision 12
# speedup vs baseline: 1.0188x; 1.0188x over previous
"""Trainium2 Bass kernel for nn_AttentionSimple (sparse_attention, 8 cores).

Reference (per batch row b):
    e      = embeddings[k[b]]              # [S, E] gather
    scores = q[b] . e[s]                   # [S]
    attn   = softmax(scores); ctx = sum_s attn[s] * e[s]
    out    = ctx @ W.T + b                 # [B, 2]

Algorithm: count-weighted vocab-space softmax — no per-token gathers.
Scores depend on s only through v = k[b, s], so group softmax terms by
vocabulary id:
    c[b, v]  = |{s : k[b, s] = v}|         (histogram of k, built on host)
    l[b, v]  = q[b] . embeddings[v]        (dense PE matmul)
    A        = c * exp(l - 30)             (global bias keeps exp in fp16;
                                            the ratio is shift-invariant)
    out[b]   = (sum_v A[b,v] * EW[v]) / (sum_v A[b,v])
    with EW  = embeddings @ W.T + b        (parameter prepacking, host)

Sharding: padded vocabulary 51200 = 400 chunks of 128, 50 chunks/core.
Each core handles all 128 batch rows for its slice; host sums the 8
partial numerators/denominators and divides.

v2 pipeline (per core), all wire data fp16 except uint8 counts:
  - et: embedding pairs packed at partition rows 0:50 / 50:100 (no
    zero-pad rows on the wire), [100, 25*128] fp16 — half the f32 bytes.
  - mm1: per pair, ps[128v, 256] = et_pair.T @ qw (qw = block-diag
    [qT|0; 0|qT] fp16, [100, 256]); 4 pairs fill a [128, 1024] 2-bank
    PSUM block (2 quads).
  - ACT: le = exp(ps - 30) fused PSUM->SBUF, fp16 out, one 1024-col
    instruction per block (amortizes the ~250ns fixed ACT overhead).
  - DVE: le *= counts (uint8, exact; max count 5) in place, 1024 cols.
  - mm2: acc[9, 512] += st9_quad.T @ le_quad (fp16, f32 PSUM accum);
    st9 = [EW_c0..EW_c3 | ones] per quad; 13 accumulating matmuls.
  - 50 chunks = 12 full quads + 1 half quad (chunks 48,49); the half
    block's unused PSUM cols never feed mm2 (rhs is narrowed to 256).
  - Warm-up matmuls ramp the PE p-state while the first DMAs land.
  - Inputs live in single resident SBUF tiles; DMAs are column-sliced
    so compute only waits on the slice it reads (subtile deps).  et
    slices ride the Sync queue, qw/st/ct ride the Scalar queue, so
    issue cost is split across two queues.
  - Output: acc PSUM is DMA'd straight to DRAM (no SBUF copy).
"""

import numpy as np

BATCH, SEQ, EMB, VOCAB, OUT = 128, 8192, 50, 50000, 2
N_CORES = 8
CSH = 50                         # vocab chunks per core
NCHUNK = CSH * N_CORES           # 400
VPAD = NCHUNK * 128              # 51200
VSH = CSH * 128                  # 6400
NPAIR = CSH // 2                 # 25
NQUAD = 13                       # 12 full + 1 half
NBLOCK = 7                       # 6 full (4 pairs) + 1 micro (1 pair)
EXP_BIAS = -30.0
NWARM = 3

_CACHE = {}


def _build_nc():
    from contextlib import ExitStack

    import concourse.mybir as mybir
    import concourse.tile as tile
    from concourse import bacc

    f32 = mybir.dt.float32
    f16 = mybir.dt.float16
    u8 = mybir.dt.uint8
    nc = bacc.Bacc("TRN2", target_bir_lowering=False, debug=False,
                   num_devices=N_CORES)

    et_d = nc.dram_tensor("et", [100, NPAIR * 128], f16, kind="ExternalInput")
    qw_d = nc.dram_tensor("qw", [100, 256], f16, kind="ExternalInput")
    st_d = nc.dram_tensor("st", [128, NQUAD * 9], f16, kind="ExternalInput")
    ct_d = nc.dram_tensor("ct", [128, VSH], u8, kind="ExternalInput")
    o_d = nc.dram_tensor("o", [18, 512], f32, kind="ExternalOutput")

    with tile.TileContext(nc) as tc, ExitStack() as ctx:
        const_p = ctx.enter_context(tc.tile_pool(name="const", bufs=1))
        ps_p = ctx.enter_context(tc.tile_pool(name="ps", bufs=3, space="PSUM"))
        acc_p = ctx.enter_context(tc.tile_pool(name="acc", bufs=1,
                                               space="PSUM"))
        le_p = ctx.enter_context(tc.tile_pool(name="le", bufs=3))

        # Two accumulators: accA (quads 0-5) finalizes mid-kernel so its
        # copy + output DMA overlap the remaining blocks; accB takes the
        # rest.  Warm-up matmuls write into accB's bank (start=True on the
        # first real accB matmul resets it), keeping PSUM within 8 banks.
        accA = acc_p.tile([9, 512], f32, tag="accA")
        accBf = acc_p.tile([128, 512], f32, tag="accB")
        accB = accBf[0:9, :]

        # PE warm-up: matmuls on a zeroed tile while input DMAs land, so
        # the PE p-state ramp starts as early as possible.
        wtile = const_p.tile([128, 512], f16)
        nc.vector.memset(wtile[:], 0.0)
        bias_sb = const_p.tile([128, 1], f32)
        nc.vector.memset(bias_sb[:], EXP_BIAS)
        for _ in range(NWARM):
            nc.tensor.matmul(accBf[:], lhsT=wtile[:, 0:128], rhs=wtile[:],
                             start=True, stop=True, skip_group_check=True)

        # Resident input tiles.  All bulk slices ride the ONE Sync queue in
        # exact consumption order — a single queue's transfers stay ordered
        # across all 16 DMA engines, so a later ct slice can never steal
        # bandwidth from an earlier et slice (the v3 regression).
        qw_sb = const_p.tile([100, 256], f16)
        st_sb = const_p.tile([128, NQUAD * 9], f16)
        nc.scalar.dma_start(st_sb[:], st_d.ap())
        et_sb = const_p.tile([100, NPAIR * 128], f16)
        ct_sb = const_p.tile([128, VSH], u8)
        nc.sync.dma_start(qw_sb[:], qw_d.ap())
        for dst, dram, c0, c1 in (
                (et_sb, et_d, 0, 1024),        # blocks 0-1 (pairs 0-7)
                (ct_sb, ct_d, 0, 2048),        # counts for blocks 0-1+
                (et_sb, et_d, 1024, 2048),     # blocks 2-3
                (ct_sb, ct_d, 2048, 4096),     # counts mid
                (et_sb, et_d, 2048, 3200),     # blocks 4-6
                (ct_sb, ct_d, 4096, VSH),      # counts tail
        ):
            nc.sync.dma_start(dst[:, c0:c1], dram.ap()[:, c0:c1])

        osbA = const_p.tile([9, 512], f32)
        osbB = const_p.tile([9, 512], f32)

        # Block structure: b0 = 1 quad (short pipeline fill), b1-b5 = 2
        # quads, b6 = q11 + half q12.  (pair, quad) offsets per block:
        BLOCKS = [(0, 0, 2), (2, 1, 4), (6, 3, 4), (10, 5, 4),
                  (14, 7, 4), (18, 9, 4), (22, 11, 3)]
        for blk, (pair0, quad0, npair) in enumerate(BLOCKS):
            ncol = 256 * npair
            ps = ps_p.tile([128, 1024], f32, tag="ps")
            for lp in range(npair):
                pair = pair0 + lp
                nc.tensor.matmul(
                    ps[:, lp * 256:(lp + 1) * 256],
                    lhsT=et_sb[:, pair * 128:(pair + 1) * 128],
                    rhs=qw_sb[:],
                    start=True, stop=True,
                )
            le = le_p.tile([128, 1024], f16, tag="le")
            nc.scalar.activation(le[:, 0:ncol], ps[:, 0:ncol],
                                 mybir.ActivationFunctionType.Exp,
                                 bias=bias_sb[:])
            nc.vector.tensor_mul(
                le[:, 0:ncol], le[:, 0:ncol],
                ct_sb[:, pair0 * 256:pair0 * 256 + ncol])
            nquad = (npair + 1) // 2
            for lq in range(nquad):
                quad = quad0 + lq
                qcol = min(512, ncol - lq * 512)
                acc = accA if quad < 6 else accB
                nc.tensor.matmul(
                    acc if qcol == 512 else acc[:, 0:qcol],
                    lhsT=st_sb[:, quad * 9:(quad + 1) * 9],
                    rhs=le[:, lq * 512:lq * 512 + qcol],
                    start=(quad in (0, 6)), stop=(quad in (5, NQUAD - 1)),
                    skip_group_check=True,
                )
            if blk == 3:
                # accA finalized (quads 0-5): ship it while blocks 4-6 run.
                nc.scalar.copy(osbA[:], accA)
                nc.sync.dma_start(o_d.ap()[0:9, :], osbA[:])

        nc.scalar.copy(osbB[:], accB)
        nc.sync.dma_start(o_d.ap()[9:18, :], osbB[:])

    nc.finalize()
    return nc


def _prep_inputs(q, k, embeddings, W, b):
    q = np.ascontiguousarray(q, dtype=np.float32)
    emb = np.ascontiguousarray(embeddings, dtype=np.float32)
    W = np.ascontiguousarray(W, dtype=np.float32)
    b = np.ascontiguousarray(b, dtype=np.float32)
    k = np.asarray(k)

    embT = np.zeros((EMB, VPAD), np.float32)
    embT[:, :VOCAB] = emb.T

    # mm1 moving operand: block-diagonal [qT | 0; 0 | qT], rows 0:50/50:100
    qw = np.zeros((100, 256), np.float16)
    qw[:EMB, 0:BATCH] = q.T
    qw[EMB:2 * EMB, BATCH:256] = q.T

    # weight prepacking: EW = emb @ W.T + b (function of parameters only)
    EWp = np.zeros((VPAD, OUT), np.float32)
    EWp[:VOCAB] = emb @ W.T + b[None, :]

    flat = (np.arange(BATCH, dtype=np.int64)[:, None] * VPAD
            + k.astype(np.int64)).ravel()
    C = np.bincount(flat, minlength=BATCH * VPAD).reshape(BATCH, VPAD)
    assert C.max() <= 255, "count histogram overflows uint8 transport"

    in_maps = []
    for core in range(N_CORES):
        v0 = core * VSH
        blocks = embT[:, v0:v0 + VSH].reshape(EMB, CSH, 128)
        e2 = np.zeros((100, NPAIR, 128), np.float16)
        e2[:EMB] = blocks[:, 0::2, :]
        e2[EMB:2 * EMB] = blocks[:, 1::2, :]
        e2 = np.ascontiguousarray(e2.reshape(100, NPAIR * 128))

        # st9 per quad: cols 2j+o = EW[chunk 4q+j, p, o]; col 8 = 1
        ew_blocks = EWp[v0:v0 + VSH].reshape(CSH, 128, OUT)  # [50, 128, 2]
        st = np.zeros((128, NQUAD, 9), np.float32)
        for quad in range(NQUAD):
            for j in range(4):
                ch = 4 * quad + j
                if ch < CSH:
                    st[:, quad, 2 * j:2 * j + 2] = ew_blocks[ch]
        st[:, :, 8] = 1.0
        st = np.ascontiguousarray(
            st.reshape(128, NQUAD * 9).astype(np.float16))

        ct = np.ascontiguousarray(
            C[:, v0:v0 + VSH].reshape(BATCH, CSH, 128)
            .transpose(2, 1, 0).reshape(128, CSH * BATCH)
            .astype(np.uint8))
        in_maps.append({"et": e2, "qw": qw, "st": st, "ct": ct})
    return in_maps


def _run_device(in_maps, **kwargs):
    from concourse.bass_utils import run_bass_kernel_spmd

    if "nc" not in _CACHE:
        _CACHE["nc"] = _build_nc()
    return run_bass_kernel_spmd(_CACHE["nc"], in_maps,
                                core_ids=list(range(N_CORES)), **kwargs)


def _unshard(res):
    P = np.zeros((9, 512), np.float64)
    for i in range(N_CORES):
        o = res.results[i]["o"].astype(np.float64)
        P += o[0:9] + o[9:18]
    numer = np.zeros((OUT, BATCH), np.float64)
    denom = np.zeros(BATCH, np.float64)
    for j in range(4):
        numer += P[2 * j:2 * j + 2, j * BATCH:(j + 1) * BATCH]
        denom += P[8, j * BATCH:(j + 1) * BATCH]
    out = (numer / denom[None, :]).T
    return np.ascontiguousarray(out, dtype=np.float32)


def kernel(q, k, embeddings, W, b, **_unused):
    in_maps = _prep_inputs(q, k, embeddings, W, b)
    res = _run_device(in_maps)
    return _unshard(res)
